# revision 20
# baseline (speedup 1.0000x reference)
"""CKA (centered kernel alignment) on 8 Trainium2 NeuronCores.

Math: with H = I - 11^T/n, H G H = (Hz)(Hz)^T, so each HSIC term is the
Frobenius norm^2 of a feature-covariance block of C = zc^T zc where
zc = [x - colmean(x) | y - colmean(y)] (8192 x 3072):
    hsic_xy = ||C[x-cols, y-cols]||_F^2   (etc.)
Column-centering happens on the HOST (exact), so the device only computes
C's upper-triangle 128x128 blocks and partial sums of squares -- no
centering pass, no column sums, and (crucially) NO collectives.

Sharding: the 24 column-tiles (128 wide) of zc form 300 unordered tile
pairs {a,b} (24 diagonal + 276 off-diagonal).  Pairs are covered by a
rotation design: core r computes blocks {(s+3r)%24, (s+3r+d)%24} for
s in {0,1,2}, d in 0..12 (312 block instances; the 12 d=12 pairs are
computed twice, weighted 1/2 on the host).  Each core therefore needs
only 15 consecutive (mod 24) column tiles -- the host pre-rotates and
packs them, so the device program is rank-uniform: fixed SBUF offsets,
different data.

Inputs are quantized to fp8e4 on the host (validated: rel-err ~3e-4 on
the final scalar vs the f64 reference; tolerance is 2e-2).  fp8 makes
the per-core panel 15.7 MB so it is fully SBUF-resident, and enables
DoubleRow matmuls (2 contraction rows per PE cell) for ~1.4x PE rate.

Performance model (trace-derived): the stream is PE-bound at ~67 us of
matmul streaming (DoubleRow fp8, 216 ns per 512-wide matmul) while the
panel DMA (~15.7 MB at roughly wire rate) finishes slightly earlier, so
phase A (s=0,1, riding the DMA window) and the resident s=2 phases run
back to back with only small supply brushes.

Device program per core:
  - panel DMAs on the Sync HWDGE queue only (the Scalar queue's issue
    is delayed by its act-table preamble): four single-pair chunks so
    t=0..3 start early, then 4-ktile chunks.  Contraction rows are
    relabeled "(p h) w" (the Gram sum is row-permutation invariant) so
    each partition reads one contiguous 3840/7680 B HBM segment.
  - 4 HAM warm-up matmuls on a zeroed tile bridge the DMA latency so
    the clock gate (1.2 -> 2.4 GHz) opens as the real stream begins.
  - LDWEIGHTS dedup (post-compile pass): the matmuls of one (t, s)
    group share the stationary tile; only the group's first Ldweights
    survives, so no weight load is exposed behind a 128-wide matmul.
  - phase A (s=0,1): t-outer chains so matmuls start as chunks land;
    its two narrow chains share one PSUM bank (7 banks total), leaving
    a free bank for phase B's first chain.  s=2 then runs on resident
    data as [(2,0),(2,1)], [(2,2)], [(2,3)] -- ending on the narrow
    chain keeps the post-stream square+DMA tail minimal.
  - per chain end: square+reduce per 128-col sub-block into acc[128,48],
    alternating ACT (fused Square+accum) and DVE (copy/mul/reduce);
    host sums partitions and applies pair weights + CKA formula in f64.
"""

import os

import numpy as np

N = 8192               # examples
NT = 24                # 128-col tiles of z = [x | y] (2048 + 1024 = 3072)
NXT = 16               # tiles belonging to x
RES = 15               # resident tiles per core (positions 0..14)
DW = RES * 128         # 1920
P = 128
KT = N // P            # 64 contraction tiles
N_CORES = 8
RUNW = (4, 4, 4, 1)    # partner-run widths per sigma (partners d = 0..12)
CHUNK = 4              # ktiles per DMA (512 rows)

_DEDUP = os.environ.get("CKA_DEDUP", "1") == "1"
_NWARM = int(os.environ.get("CKA_NWARM", "6"))

_COMPILED = None


def _dedup_ldweights(nc, mybir):
    """Drop InstLdweights that reload the exact weights already resident.

    The bacc codegen splits every InstMatmult into Ldweights+Matmult even
    when consecutive matmuls share the stationary tile.  A redundant
    Ldweights (identical weights AP, no semaphore waits/updates) after a
    short 128-wide matmul is exposed on the PE queue (~136 ns load vs
    ~56 ns stream) and stalls the stream.  Deleting the duplicates is
    safe: waits were already hoisted onto the group's first Ldweights,
    and the matmuls (all ldweights=False post-split) use the foreground
    weights the kept Ldweights loaded.
    """
    n_del = 0
    for b in nc.main_func.blocks:
        insts = b.instructions
        out, rename = [], {}
        last_sig, last_name = None, None
        for i in insts:
            if isinstance(i, mybir.InstLdweights):
                sig = (str(i.ins[0]), str(i.perf_mode), str(i.tile_size),
                       str(i.tile_position), str(i.is_transpose))
                if (sig == last_sig and not i.has_wait()
                        and not i.has_update()):
                    rename[i.name] = last_name
                    n_del += 1
                    continue
                last_sig, last_name = sig, i.name
            out.append(i)
        if rename:
            for i in out:
                i.remap_dependency_names(rename)
            b.instructions = out
    return n_del


def _build():
    import concourse.bacc as bacc
    import concourse.mybir as mybir
    import concourse.tile as tile

    f32 = mybir.dt.float32
    f8 = mybir.dt.float8e4

    nc = bacc.Bacc("TRN2", target_bir_lowering=False, debug=False,
                   num_devices=N_CORES)
    z = nc.dram_tensor("z", [N, DW], f8, kind="ExternalInput")
    out = nc.dram_tensor("partials", [P, 48], f32, kind="ExternalOutput")

    with tile.TileContext(nc) as tc:
        with (
            tc.tile_pool(name="persist", bufs=1) as persist,
            tc.tile_pool(name="spill", bufs=2) as spill,
            tc.tile_pool(name="psum", bufs=8, space="PSUM") as psum,
        ):
            zb = persist.tile([P, KT, DW], f8)
            # Chunked panel stream, rows relabeled "(p h) w" so each
            # partition reads one contiguous HBM segment.  The first four
            # chunks are single DoubleRow pairs so t=0..3 start as soon
            # as possible; everything is issued in order on the Sync
            # HWDGE queue so chunks complete in consumption order (the
            # Scalar queue's issue is delayed by its act-table preamble).
            chunks = [2, 2, 2, 2] + [CHUNK] * ((KT - 8) // CHUNK)
            k0 = 0
            for ck in chunks:
                nc.sync.dma_start(
                    zb[:, k0:k0 + ck, :],
                    z[k0 * P:(k0 + ck) * P, :].rearrange(
                        "(p h) w -> p h w", p=P))
                k0 += ck

            acc = persist.tile([P, 48], f32)
            nc.vector.memset(acc[:], 0.0)

            # HAM warm-up: dummy matmuls on a zeroed tile keep the PE
            # busy while the first DMA lands, so the clock gate opens
            # (1.2 -> 2.4 GHz) close to when the real stream begins.
            # The memset runs on GpSimd, which clears its preamble ~1 us
            # before Vector, so warm-ups start early enough to open the
            # clock gate right as the first chunk lands.
            zw = persist.tile([P, 2, 512], f8)
            nc.gpsimd.memset(zw[:], 0.0)
            psw = psum.tile([P, 512], f32, tag="ps", name="ps")
            for _ in range(_NWARM):
                nc.tensor.matmul(
                    psw[:], zw[:, :, 0:128], zw[:, :, :],
                    start=True, stop=True,
                    perf_mode=mybir.MatmulPerfMode.DoubleRow)
            junkw = spill.tile([P, P], f32, tag="junk", name="junk",
                               bufs=8)
            nc.scalar.copy(junkw[:], psw[:, 0:P])

            def phase(slots, shared_narrow=False, lag=0):
                pss = {}
                psn = None
                for s, j in slots:
                    if shared_narrow and j == 3:
                        if psn is None:
                            psn = psum.tile([P, 512], f32, tag="ps",
                                            name="ps")
                        pss[s, j] = psn[:, 128 * s:128 * (s + 1)]
                    else:
                        pss[s, j] = psum.tile([P, 512], f32, tag="ps",
                                              name="ps")
                for tt in range(KT // 2 + lag * (len(slots) - 1)):
                    for n_sl, (s, j) in enumerate(slots):
                        t = tt - lag * n_sl
                        if not (0 <= t < KT // 2):
                            continue
                        w = RUNW[j] * P
                        c0 = (s + 4 * j) * P
                        lhs = zb[:, 2 * t:2 * t + 2,
                                 s * P:(s + 1) * P]
                        rhs = zb[:, 2 * t:2 * t + 2, c0:c0 + w]
                        nc.tensor.matmul(
                            pss[s, j][:, 0:w], lhs, rhs,
                            start=(t == 0), stop=(t == KT // 2 - 1),
                            perf_mode=mybir.MatmulPerfMode.DoubleRow)
                # square + reduce each 128-col sub-block into its acc column,
                # alternating ACT (fused square+accum) and DVE (mul+reduce)
                # so consecutive squares run on two engines in parallel.
                # Narrow (j=3) slots drain first: they share one PSUM bank,
                # and the next phase's second chain reuses it -- draining
                # it last would stall that chain ~6 us into the phase.
                sq_order = ([sl for sl in slots if sl[1] == 3]
                            + [sl for sl in slots if sl[1] != 3])
                n_sq = 0
                for s, j in sq_order:
                    for t4 in range(RUNW[j]):
                        ps = pss[s, j][:, t4 * P:(t4 + 1) * P]
                        junk = spill.tile([P, P], f32, tag="junk",
                                          name="junk", bufs=8)
                        col = s * 16 + j * 4 + t4
                        if n_sq % 2 == 0:
                            nc.scalar.activation(
                                junk[:], ps,
                                mybir.ActivationFunctionType.Square,
                                accum_out=acc[:, col:col + 1])
                        else:
                            nc.vector.tensor_copy(junk[:], ps)
                            sq = spill.tile([P, P], f32, tag="junk",
                                            name="junk", bufs=8)
                            nc.vector.tensor_mul(sq[:], junk[:], junk[:])
                            nc.vector.tensor_reduce(
                                out=acc[:, col:col + 1], in_=sq[:],
                                axis=mybir.AxisListType.X,
                                op=mybir.AluOpType.add)
                        n_sq += 1

            # Phase A (s=0,1) rides the DMA window; its two narrow
            # chains share one PSUM bank so a bank is free for phase B's
            # first chain the moment A ends.  Group order keeps each
            # LDWEIGHTS behind a 512-wide matmul.  s=2 then runs on
            # resident data, ending with the single-subcol w1 chain so
            # the post-stream square+DMA tail is minimal.
            phase([(0, 0), (0, 3), (0, 1), (0, 2),
                   (1, 0), (1, 3), (1, 1), (1, 2)], shared_narrow=True)
            nc.sync.dma_start(out[:, 0:32], acc[:, 0:32])
            # stagger (2,1) 8 t-steps behind (2,0) so its first matmul
            # does not head-of-line-block the PE while phase A's first
            # square group drains the PSUM bank it reuses
            phase([(2, 0), (2, 1)], lag=8)
            phase([(2, 2)])
            phase([(2, 3)])
            nc.sync.dma_start(out[:, 32:48], acc[:, 32:48])

    nc.compile()
    if _DEDUP:
        _dedup_ldweights(nc, mybir)
    return nc


def _get_compiled():
    global _COMPILED
    if _COMPILED is None:
        _COMPILED = _build()
    return _COMPILED


def _pack_inputs(x, y):
    """Center columns, quantize to fp8e4, build each core's rotated panel."""
    import ml_dtypes
    x = np.asarray(x)
    y = np.asarray(y)
    xc = (x - x.mean(axis=0, dtype=np.float64).astype(np.float32))
    yc = (y - y.mean(axis=0, dtype=np.float64).astype(np.float32))
    xq = xc.astype(ml_dtypes.float8_e4m3)
    yq = yc.astype(ml_dtypes.float8_e4m3)
    tiles = ([xq[:, c * P:(c + 1) * P] for c in range(NXT)]
             + [yq[:, c * P:(c + 1) * P] for c in range(NT - NXT)])
    in_maps = []
    for r in range(N_CORES):
        cols = [(3 * r + p) % NT for p in range(RES)]
        zr = np.ascontiguousarray(
            np.concatenate([tiles[c] for c in cols], axis=1))
        in_maps.append({"z": zr})
    return in_maps


def _combine(partials):
    """Host reduction: weighted sums of per-block ssq -> CKA scalar."""
    hxx = hxy = hyy = 0.0
    for r in range(N_CORES):
        p = np.asarray(partials[r], dtype=np.float64)
        colsums = p.sum(axis=0)
        for s in range(3):
            a = (3 * r + s) % NT
            for j in range(4):
                for t4 in range(RUNW[j]):
                    d = 4 * j + t4
                    b = (3 * r + s + d) % NT
                    ssq = colsums[s * 16 + j * 4 + t4]
                    cov = 2.0 if d == 12 else 1.0
                    ax, bx = a < NXT, b < NXT
                    if ax and bx:
                        hxx += (1.0 if d == 0 else 2.0) / cov * ssq
                    elif not ax and not bx:
                        hyy += (1.0 if d == 0 else 2.0) / cov * ssq
                    else:
                        hxy += 1.0 / cov * ssq
    return np.float32(hxy / (np.sqrt(hxx * hyy) + 1e-8))


def _run(x, y, trace=False):
    import time
    from concourse import bass_utils
    nc = _get_compiled()
    in_maps = _pack_inputs(x, y)
    last_err = None
    for attempt in range(3):
        try:
            res = bass_utils.run_bass_kernel_spmd(
                nc, in_maps, core_ids=list(range(N_CORES)), trace=trace)
            break
        except Exception as e:  # transient device wedge: retry
            last_err = e
            time.sleep(5.0)
    else:
        raise last_err
    val = _combine([res.results[r]["partials"] for r in range(N_CORES)])
    return np.asarray(val, dtype=np.float32), res


def kernel(x, y):
    val, _ = _run(x, y, trace=False)
    return val


# revision 22
# speedup vs baseline: 1.0108x; 1.0108x over previous
"""CKA (centered kernel alignment) on 8 Trainium2 NeuronCores.

Math: with H = I - 11^T/n, H G H = (Hz)(Hz)^T, so each HSIC term is the
Frobenius norm^2 of a feature-covariance block of C = zc^T zc where
zc = [x - colmean(x) | y - colmean(y)] (8192 x 3072):
    hsic_xy = ||C[x-cols, y-cols]||_F^2   (etc.)
Column-centering happens on the HOST (exact), so the device only computes
C's upper-triangle 128x128 blocks and partial sums of squares -- no
centering pass, no column sums, and (crucially) NO collectives.

Sharding: the 24 column-tiles (128 wide) of zc form 300 unordered tile
pairs {a,b} (24 diagonal + 276 off-diagonal).  Pairs are covered by a
rotation design: core r computes blocks {(s+3r)%24, (s+3r+d)%24} for
s in {0,1,2}, d in 0..12 (312 block instances; the 12 d=12 pairs are
computed twice, weighted 1/2 on the host).  Each core therefore needs
only 15 consecutive (mod 24) column tiles -- the host pre-rotates and
packs them, so the device program is rank-uniform: fixed SBUF offsets,
different data.

Inputs are quantized to fp8e4 on the host (validated: rel-err ~3e-4 on
the final scalar vs the f64 reference; tolerance is 2e-2).  fp8 makes
the per-core panel 15.7 MB so it is fully SBUF-resident, and enables
DoubleRow matmuls (2 contraction rows per PE cell) for ~1.4x PE rate.

Performance model (trace-derived): the stream is PE-bound at ~67 us of
matmul streaming (DoubleRow fp8, 216 ns per 512-wide matmul) while the
panel DMA (~15.7 MB at roughly wire rate) finishes slightly earlier, so
phase A (s=0,1, riding the DMA window) and the resident s=2 phases run
back to back with only small supply brushes.

Device program per core:
  - panel DMAs on the Sync HWDGE queue only (the Scalar queue's issue
    is delayed by its act-table preamble): four single-pair chunks so
    t=0..3 start early, then 4-ktile chunks.  Contraction rows are
    relabeled "(p h) w" (the Gram sum is row-permutation invariant) so
    each partition reads one contiguous 3840/7680 B HBM segment.
  - 6 HAM warm-up matmuls on a zeroed tile (memset on GpSimd, whose
    queue clears its preamble ~1 us before Vector) bridge the DMA
    latency so the clock gate (1.2 -> 2.4 GHz) opens as the real
    stream begins.
  - LDWEIGHTS dedup (post-compile pass): the matmuls of one (t, s)
    group share the stationary tile; only the group's first Ldweights
    survives, so no weight load is exposed behind a 128-wide matmul.
  - phase A (s=0,1): t-outer chains so matmuls start as chunks land;
    its two narrow chains share one PSUM bank (7 banks total), leaving
    a free bank for phase B's first chain.  s=2 then runs on resident
    data as [(2,0),(2,1) lagged 8 t-steps], [(2,2)], [(2,3)] -- ending
    on the narrow chain keeps the post-stream square+DMA tail minimal.
    Narrow slots' squares drain first so the shared bank is available
    when the next phase's lagged chain reaches for it.
  - per chain end: square+reduce per 128-col sub-block into acc[128,48],
    alternating ACT (fused Square+accum) and DVE (copy/mul/reduce);
    host sums partitions and applies pair weights + CKA formula in f64.
"""

import os

import numpy as np

N = 8192               # examples
NT = 24                # 128-col tiles of z = [x | y] (2048 + 1024 = 3072)
NXT = 16               # tiles belonging to x
RES = 15               # resident tiles per core (positions 0..14)
DW = RES * 128         # 1920
P = 128
KT = N // P            # 64 contraction tiles
N_CORES = 8
RUNW = (4, 4, 4, 1)    # partner-run widths per sigma (partners d = 0..12)
CHUNK = 4              # ktiles per DMA (512 rows)

_DEDUP = os.environ.get("CKA_DEDUP", "1") == "1"
_NWARM = int(os.environ.get("CKA_NWARM", "7"))

_COMPILED = None


def _dedup_ldweights(nc, mybir):
    """Drop InstLdweights that reload the exact weights already resident.

    The bacc codegen splits every InstMatmult into Ldweights+Matmult even
    when consecutive matmuls share the stationary tile.  A redundant
    Ldweights (identical weights AP, no semaphore waits/updates) after a
    short 128-wide matmul is exposed on the PE queue (~136 ns load vs
    ~56 ns stream) and stalls the stream.  Deleting the duplicates is
    safe: waits were already hoisted onto the group's first Ldweights,
    and the matmuls (all ldweights=False post-split) use the foreground
    weights the kept Ldweights loaded.
    """
    n_del = 0
    for b in nc.main_func.blocks:
        insts = b.instructions
        out, rename = [], {}
        last_sig, last_name = None, None
        for i in insts:
            if isinstance(i, mybir.InstLdweights):
                sig = (str(i.ins[0]), str(i.perf_mode), str(i.tile_size),
                       str(i.tile_position), str(i.is_transpose))
                if (sig == last_sig and not i.has_wait()
                        and not i.has_update()):
                    rename[i.name] = last_name
                    n_del += 1
                    continue
                last_sig, last_name = sig, i.name
            out.append(i)
        if rename:
            for i in out:
                i.remap_dependency_names(rename)
            b.instructions = out
    return n_del


def _build():
    import concourse.bacc as bacc
    import concourse.mybir as mybir
    import concourse.tile as tile

    f32 = mybir.dt.float32
    f8 = mybir.dt.float8e4

    nc = bacc.Bacc("TRN2", target_bir_lowering=False, debug=False,
                   num_devices=N_CORES)
    z = nc.dram_tensor("z", [N, DW], f8, kind="ExternalInput")
    out = nc.dram_tensor("partials", [P, 48], f32, kind="ExternalOutput")

    with tile.TileContext(nc) as tc:
        with (
            tc.tile_pool(name="persist", bufs=1) as persist,
            tc.tile_pool(name="spill", bufs=2) as spill,
            tc.tile_pool(name="psum", bufs=8, space="PSUM") as psum,
        ):
            zb = persist.tile([P, KT, DW], f8)
            # Chunked panel stream, rows relabeled "(p h) w" so each
            # partition reads one contiguous HBM segment.  The first four
            # chunks are single DoubleRow pairs so t=0..3 start as soon
            # as possible; everything is issued in order on the Sync
            # HWDGE queue so chunks complete in consumption order (the
            # Scalar queue's issue is delayed by its act-table preamble).
            chunks = [2, 2, 2, 2] + [CHUNK] * ((KT - 8) // CHUNK)
            k0 = 0
            for ck in chunks:
                nc.sync.dma_start(
                    zb[:, k0:k0 + ck, :],
                    z[k0 * P:(k0 + ck) * P, :].rearrange(
                        "(p h) w -> p h w", p=P))
                k0 += ck

            acc = persist.tile([P, 48], f32)
            nc.vector.memset(acc[:], 0.0)

            # HAM warm-up: dummy matmuls on a zeroed tile keep the PE
            # busy while the first DMA lands, so the clock gate opens
            # (1.2 -> 2.4 GHz) close to when the real stream begins.
            # The memset runs on GpSimd, which clears its preamble ~1 us
            # before Vector, so warm-ups start early enough to open the
            # clock gate right as the first chunk lands.
            zw = persist.tile([P, 2, 512], f8)
            nc.gpsimd.memset(zw[:], 0.0)
            psw = psum.tile([P, 512], f32, tag="ps", name="ps")
            for _ in range(_NWARM):
                nc.tensor.matmul(
                    psw[:], zw[:, :, 0:128], zw[:, :, :],
                    start=True, stop=True,
                    perf_mode=mybir.MatmulPerfMode.DoubleRow)
            junkw = spill.tile([P, P], f32, tag="junk", name="junk",
                               bufs=8)
            nc.scalar.copy(junkw[:], psw[:, 0:P])

            def phase(slots, shared_narrow=False, lag=0):
                pss = {}
                psn = None
                for s, j in slots:
                    if shared_narrow and j == 3:
                        if psn is None:
                            psn = psum.tile([P, 512], f32, tag="ps",
                                            name="ps")
                        pss[s, j] = psn[:, 128 * s:128 * (s + 1)]
                    else:
                        pss[s, j] = psum.tile([P, 512], f32, tag="ps",
                                              name="ps")
                for tt in range(KT // 2 + lag * (len(slots) - 1)):
                    for n_sl, (s, j) in enumerate(slots):
                        t = tt - lag * n_sl
                        if not (0 <= t < KT // 2):
                            continue
                        w = RUNW[j] * P
                        c0 = (s + 4 * j) * P
                        lhs = zb[:, 2 * t:2 * t + 2,
                                 s * P:(s + 1) * P]
                        rhs = zb[:, 2 * t:2 * t + 2, c0:c0 + w]
                        nc.tensor.matmul(
                            pss[s, j][:, 0:w], lhs, rhs,
                            start=(t == 0), stop=(t == KT // 2 - 1),
                            perf_mode=mybir.MatmulPerfMode.DoubleRow)
                # square + reduce each 128-col sub-block into its acc column,
                # alternating ACT (fused square+accum) and DVE (mul+reduce)
                # so consecutive squares run on two engines in parallel.
                # Narrow (j=3) slots drain first: they share one PSUM bank,
                # and the next phase's second chain reuses it -- draining
                # it last would stall that chain ~6 us into the phase.
                sq_order = ([sl for sl in slots if sl[1] == 3]
                            + [sl for sl in slots if sl[1] != 3])
                n_sq = 0
                for s, j in sq_order:
                    for t4 in range(RUNW[j]):
                        ps = pss[s, j][:, t4 * P:(t4 + 1) * P]
                        junk = spill.tile([P, P], f32, tag="junk",
                                          name="junk", bufs=8)
                        col = s * 16 + j * 4 + t4
                        if n_sq % 2 == 0:
                            nc.scalar.activation(
                                junk[:], ps,
                                mybir.ActivationFunctionType.Square,
                                accum_out=acc[:, col:col + 1])
                        else:
                            nc.vector.tensor_copy(junk[:], ps)
                            sq = spill.tile([P, P], f32, tag="junk",
                                            name="junk", bufs=8)
                            nc.vector.tensor_mul(sq[:], junk[:], junk[:])
                            nc.vector.tensor_reduce(
                                out=acc[:, col:col + 1], in_=sq[:],
                                axis=mybir.AxisListType.X,
                                op=mybir.AluOpType.add)
                        n_sq += 1

            # Phase A (s=0,1) rides the DMA window; its two narrow
            # chains share one PSUM bank so a bank is free for phase B's
            # first chain the moment A ends.  Group order keeps each
            # LDWEIGHTS behind a 512-wide matmul.  s=2 then runs on
            # resident data, ending with the single-subcol w1 chain so
            # the post-stream square+DMA tail is minimal.
            phase([(0, 0), (0, 3), (0, 1), (0, 2),
                   (1, 0), (1, 3), (1, 1), (1, 2)], shared_narrow=True)
            nc.sync.dma_start(out[:, 0:32], acc[:, 0:32])
            # stagger (2,1) 8 t-steps behind (2,0) so its first matmul
            # does not head-of-line-block the PE while phase A's first
            # square group drains the PSUM bank it reuses
            phase([(2, 0), (2, 1)], lag=8)
            phase([(2, 2)])
            phase([(2, 3)])
            nc.sync.dma_start(out[:, 32:48], acc[:, 32:48])

    nc.compile()
    if _DEDUP:
        _dedup_ldweights(nc, mybir)
    return nc


def _get_compiled():
    global _COMPILED
    if _COMPILED is None:
        _COMPILED = _build()
    return _COMPILED


def _pack_inputs(x, y):
    """Center columns, quantize to fp8e4, build each core's rotated panel."""
    import ml_dtypes
    x = np.asarray(x)
    y = np.asarray(y)
    xc = (x - x.mean(axis=0, dtype=np.float64).astype(np.float32))
    yc = (y - y.mean(axis=0, dtype=np.float64).astype(np.float32))
    xq = xc.astype(ml_dtypes.float8_e4m3)
    yq = yc.astype(ml_dtypes.float8_e4m3)
    tiles = ([xq[:, c * P:(c + 1) * P] for c in range(NXT)]
             + [yq[:, c * P:(c + 1) * P] for c in range(NT - NXT)])
    in_maps = []
    for r in range(N_CORES):
        cols = [(3 * r + p) % NT for p in range(RES)]
        zr = np.ascontiguousarray(
            np.concatenate([tiles[c] for c in cols], axis=1))
        in_maps.append({"z": zr})
    return in_maps


def _combine(partials):
    """Host reduction: weighted sums of per-block ssq -> CKA scalar."""
    hxx = hxy = hyy = 0.0
    for r in range(N_CORES):
        p = np.asarray(partials[r], dtype=np.float64)
        colsums = p.sum(axis=0)
        for s in range(3):
            a = (3 * r + s) % NT
            for j in range(4):
                for t4 in range(RUNW[j]):
                    d = 4 * j + t4
                    b = (3 * r + s + d) % NT
                    ssq = colsums[s * 16 + j * 4 + t4]
                    cov = 2.0 if d == 12 else 1.0
                    ax, bx = a < NXT, b < NXT
                    if ax and bx:
                        hxx += (1.0 if d == 0 else 2.0) / cov * ssq
                    elif not ax and not bx:
                        hyy += (1.0 if d == 0 else 2.0) / cov * ssq
                    else:
                        hxy += 1.0 / cov * ssq
    return np.float32(hxy / (np.sqrt(hxx * hyy) + 1e-8))


def _run(x, y, trace=False):
    import time
    from concourse import bass_utils
    nc = _get_compiled()
    in_maps = _pack_inputs(x, y)
    last_err = None
    for attempt in range(3):
        try:
            res = bass_utils.run_bass_kernel_spmd(
                nc, in_maps, core_ids=list(range(N_CORES)), trace=trace)
            break
        except Exception as e:  # transient device wedge: retry
            last_err = e
            time.sleep(5.0)
    else:
        raise last_err
    val = _combine([res.results[r]["partials"] for r in range(N_CORES)])
    return np.asarray(val, dtype=np.float32), res


def kernel(x, y):
    val, _ = _run(x, y, trace=False)
    return val
